# revision 45
# baseline (speedup 1.0000x reference)
"""BinaryLinear on 8 TRN2 NeuronCores, two-phase.

Computes y = sign(x) @ sign(W)^T + bias for x [8192, 4096] fp32,
W [4096, 4096] fp32, bias [4096] fp32 -> y [8192, 4096] fp32.

Phase A (prep, one SPMD launch): core i reads W rows [i*512, (i+1)*512)
(8.4 MB fp32, 1/8 of W), signs them on ScalarE to fp8 +-1, PE-transposes
the fp8 byte pairs (bf16 u16 trick) and writes the DoubleRowSwInterleave
stationary layout swt [128, 4, 16, 128] u16 (2.1 MB) back to DRAM.
Globally W fp32 is read once (67.2 MB total vs 268.4 MB if every core
read it), which is what beats the single-launch 67.2 MB/core floor.

Host glue between launches is layout-only: concatenate the 8 swt shards
along the out-feature block axis (no arithmetic on element values).

Phase B (main, one SPMD launch): batch-parallel. Core i reads its x rows
[1024, 4096] fp32 (16.8 MB), the full packed sign-W [128, 32, 16, 128]
u16 (16.8 MB), bias, and writes out^T [4096, 1024] fp16 (8.4 MB): 42 MB
of DMA (117 us at the simulated 360 GB/s) against ~116 us of PE (2048
fp8 DoubleRow matmuls + x transposes), i.e. at the ridge:
  - wt ot-tiles stream through a small pool (one [128, 16, 128] u16 DMA
    per 128 out-features, 4 KB/partition contiguous), with x sub-tiles
    0-3 staged in the head so early matmul units always have both a
    weight tile and a batch chunk (the early phase is supply-limited:
    units need the product of arrivals from both streams).
  - x sub-tiles (128 rows) load in column quarters, each signed on
    ScalarE as it lands; PE u16-pair transposes + DVE copies build
    xTp [128, 16, 256] per 256-batch chunk.
  - one 16-matmul DoubleRow chain per (ot, chunk) unit accumulates a
    [128, 256] psum tile (wide matmuls: 128-col chains double the PE
    instruction count for the same FLOPs), then one VectorE bias-add
    converts to fp16; two chunks share one [128, 512] store (1 KB rows)
    issued on the ACT queue so stores never park the SP load stream.
Each K=4096 dot is an even integer in [-4096, 4096]; all are exactly
representable in fp16, so fp16 out + fp32 bias loses only the final
rounding (~1e-4 rel).

The host unshard transposes each core's out^T back and stacks rows.
"""

from contextlib import ExitStack

import numpy as np

import concourse.bacc as bacc
import concourse.bass as bass
import concourse.mybir as mybir
import concourse.tile as tile
from concourse import masks
from concourse.bass_utils import run_bass_kernel_spmd

F32 = mybir.dt.float32
F16 = mybir.dt.float16
U16 = mybir.dt.uint16
BF16 = mybir.dt.bfloat16
FP8 = mybir.dt.float8e4
AF = mybir.ActivationFunctionType
ALU = mybir.AluOpType

B, IN, OUT = 8192, 4096, 4096
N_CORES = 8
Bs = B // N_CORES            # 1024 batch rows per core (phase B)
WR = OUT // N_CORES          # 512 W rows per core (phase A)
K = IN
T = K // 256                 # 16 DoubleRow pair-groups
OT = OUT // 128              # 32 out-feature blocks
OTL = WR // 128              # 4 local out blocks in phase A
BC = 256                     # phase B batch chunk (2 sub-tiles)
NCH = Bs // BC               # 4 chunks
SUB = Bs // 128              # 8 x sub-tiles
KH = K // 2


def _build_prep():
    """Phase A: w [512, 4096] f32 -> swt [128, 4, 16, 128] u16."""
    nc = bacc.Bacc("TRN2", target_bir_lowering=False, debug=False)
    w = nc.declare_dram_parameter("w", [WR, K], F32, isOutput=False)
    swt = nc.declare_dram_parameter("swt", [128, OTL, T, 128], U16, isOutput=True)

    with tile.TileContext(nc) as tc, ExitStack() as ctx:
        const = ctx.enter_context(tc.tile_pool(name="const", bufs=1))
        wstage = ctx.enter_context(tc.tile_pool(name="wstage", bufs=4))
        sw_pool = ctx.enter_context(tc.tile_pool(name="sw", bufs=4))
        ptr_pool = ctx.enter_context(
            tc.tile_pool(name="ptr", bufs=2, space=bass.MemorySpace.PSUM)
        )
        out_pool = ctx.enter_context(tc.tile_pool(name="out", bufs=2))

        ident16 = const.tile([128, 128], BF16)
        masks.make_identity(nc, ident16[:])

        def load_sign_tile(ot):
            """[SP+ACT] load one 128-row W tile in two column halves, each
            signed as soon as it lands (shortens the last tile's tail)."""
            sw = sw_pool.tile([128, K], FP8, tag="sw", name="sw")
            QW = KH // 2
            for hh in range(2):
                ws = wstage.tile([128, KH], F32, tag="ws", name="ws")
                nc.sync.dma_start(
                    ws[:], w[ot * 128:(ot + 1) * 128, hh * KH:(hh + 1) * KH]
                )
                for qq in range(2):
                    nc.scalar.activation(
                        sw[:, (2 * hh + qq) * QW:(2 * hh + qq + 1) * QW],
                        ws[:, qq * QW:(qq + 1) * QW],
                        AF.Sign,
                    )
            return sw

        def trans_store(ot, sw):
            sw16 = sw.bitcast(BF16)        # [128, K//2]
            ob = out_pool.tile([128, T, 128], BF16, tag="ob", name="ob")
            for tg in range(T // 8):
                ptx = ptr_pool.tile([128, 8 * 128], BF16, tag="ptr", name="ptx")
                for j in range(8):
                    t = tg * 8 + j
                    nc.tensor.transpose(
                        ptx[:, j * 128:(j + 1) * 128],
                        sw16[:, t * 128:(t + 1) * 128],
                        ident16[:],
                    )
                # column-reversed copy (SwInterleave stationary layout)
                nc.vector.tensor_copy(
                    ob[:, tg * 8:(tg + 1) * 8, :][:, :, ::-1], ptx[:]
                )
                # ship each t-half as soon as its copy lands
                nc.sync.dma_start(
                    swt[:, ot, tg * 8:(tg + 1) * 8, :],
                    ob[:, tg * 8:(tg + 1) * 8, :].bitcast(U16),
                )

        # all 4 tiles' loads+signs issued up front (dedicated buffers),
        # transpose/store chase the DMA stream
        staged = {ot: load_sign_tile(ot) for ot in range(OTL)}
        for ot in range(OTL):
            trans_store(ot, staged.pop(ot))

    nc.compile()
    return nc


def _build_main():
    """Phase B: x [1024, 4096] f32 + wt [128, 32, 16, 128] u16 +
    b [128, 32] f32 (host-transposed bias) -> out^T [4096, 1024] f16."""
    nc = bacc.Bacc("TRN2", target_bir_lowering=False, debug=False)
    x = nc.declare_dram_parameter("x", [Bs, K], F32, isOutput=False)
    wt = nc.declare_dram_parameter("wt", [128, OT, T, 128], U16, isOutput=False)
    # bias arrives host-transposed as [128, 32] so its load is one
    # contiguous 128-descriptor DMA (the [4096] gather layout cost 1.8 us
    # of 4096 tiny descriptors at the very head of the DMA stream)
    b = nc.declare_dram_parameter("b", [128, OT], F32, isOutput=False)
    out = nc.declare_dram_parameter("out", [OUT, Bs], F16, isOutput=True)

    with tile.TileContext(nc) as tc, ExitStack() as ctx:
        const = ctx.enter_context(tc.tile_pool(name="const", bufs=1))
        wt_pool = ctx.enter_context(tc.tile_pool(name="wt", bufs=26))
        xstage = ctx.enter_context(tc.tile_pool(name="xstage", bufs=5))
        sx_pool = ctx.enter_context(tc.tile_pool(name="sx", bufs=5))
        xt_pool = ctx.enter_context(tc.tile_pool(name="xt", bufs=4))
        ptrx_pool = ctx.enter_context(
            tc.tile_pool(name="ptrx", bufs=2, space=bass.MemorySpace.PSUM)
        )
        pacc_pool = ctx.enter_context(
            tc.tile_pool(name="pacc", bufs=6, space=bass.MemorySpace.PSUM)
        )
        # all 64 paired out tiles can sit in SBUF: stores are deferred past
        # the load phase so loads never share early DMA bandwidth
        outsb = ctx.enter_context(tc.tile_pool(name="outsb", bufs=28))

        ident16 = const.tile([128, 128], BF16)
        masks.make_identity(nc, ident16[:])

        bias_sb = const.tile([128, OT], F32)

        def load_wt(ot):
            """[SP] one out-feature block of packed sign-W."""
            wtile = wt_pool.tile([128, T, 128], BF16, tag="wt", name="wtile")
            nc.sync.dma_start(wtile[:].bitcast(U16), wt[:, ot, :, :])
            return wtile

        QK = K // 4

        def load_sign_x(s):
            """[SP+ACT] one 128-row x sub-tile in four column quarters,
            each signed as soon as it lands (keeps ACT chasing the DMA)."""
            sx = sx_pool.tile([128, K], FP8, tag="sx", name="sx")
            for q in range(4):
                xs = xstage.tile([128, QK], F32, tag="xs", name="xs")
                nc.sync.dma_start(
                    xs[:], x[s * 128:(s + 1) * 128, q * QK:(q + 1) * QK]
                )
                nc.scalar.activation(sx[:, q * QK:(q + 1) * QK], xs[:], AF.Sign)
            return sx

        def alloc_xtp():
            return xt_pool.tile([128, T, BC], BF16, tag="xTp", name="xTp")

        def trans_x(sx, xTp, h):
            """[PE+DVE] u16-pair transposes of sub-tile h into xTp half h."""
            sx16 = sx.bitcast(BF16)        # [128, K//2]
            for tg in range(T // 8):
                ptx = ptrx_pool.tile([128, 8 * 128], BF16, tag="ptrx", name="ptx")
                for j in range(8):
                    t = tg * 8 + j
                    nc.tensor.transpose(
                        ptx[:, j * 128:(j + 1) * 128],
                        sx16[:, t * 128:(t + 1) * 128],
                        ident16[:],
                    )
                nc.vector.tensor_copy(
                    xTp[:, tg * 8:(tg + 1) * 8, h * 128:(h + 1) * 128], ptx[:]
                )

        def mm_unit(wtile, xTp, pacc):
            """[PE] one 16-matmul chain over the full 256-col chunk. Fewer,
            wider matmuls: the PE sequencer (~19 ns/inst) paces the kernel
            if the chunk is split into 128-col chains."""
            xTp8 = xTp.bitcast(FP8)        # [128, T, 2*BC]
            for t in range(T):
                rhs = xTp8[:, t, :].rearrange("p (b h) -> p h b", h=2)
                nc.tensor.matmul(
                    pacc[:],
                    wtile[:, t, :].bitcast(FP8),
                    rhs,
                    start=(t == 0),
                    stop=(t == T - 1),
                    perf_mode=mybir.MatmulPerfMode.DoubleRowSwInterleave,
                )

        def bias_add(ot, c, pacc, osb):
            """[DVE] bias add -> fp16 into this chunk's half of osb."""
            nc.vector.tensor_scalar(
                osb[:, (c % 2) * BC:(c % 2 + 1) * BC],
                pacc[:],
                bias_sb[:, ot:ot + 1],
                None,
                ALU.add,
            )

        def store(ot, cp, osb, queue=None):
            """[ACT] one [128, 512] fp16 store (two chunks, 1KB rows);
            issued on the ACT queue so it can never sit behind a parked
            wt load on SP."""
            (queue or nc.scalar).dma_start(
                out[ot * 128:(ot + 1) * 128, cp * 2 * BC:(cp + 1) * 2 * BC],
                osb[:],
            )

        # ---- software-pipelined slot schedule ----
        # Offline makespan search says: stage x sub-tiles 0-3 up front
        # (chunks 0-1 usable immediately), then stream wt tiles hard with
        # the remaining x sub-tiles spread out. PE demand early is supplied
        # by (chunks 0-1) x (many wt tiles) instead of starving at
        # 1 unit per wt arrival.
        xload_slot = {4: 0, 5: 1, 6: 2, 7: 3}
        loads_at, trs_at = {}, {}
        x_ready = {0: 1, 1: 1, 2: 1, 3: 1}  # head sub-tiles, transposed slot 0
        for s, sl in xload_slot.items():
            loads_at.setdefault(sl, []).append(s)
            trs_at.setdefault(sl + 1, []).append(s)
            x_ready[s] = sl + 2

        # wt: 1 in the head, then 3 per slot until ot 15, 2 per slot after —
        # front-loaded so early units always have a resident wt tile.
        loads_wt_at = {}
        slot_budget = [3] * 5 + [2] * 40
        nxt = 1
        sl = 0
        while nxt < OT:
            take = min(slot_budget[sl], OT - nxt)
            for _ in range(take):
                loads_wt_at.setdefault(sl, []).append(nxt)
                nxt += 1
            sl += 1

        NSLOT = OT + 4
        STORE_START = 2
        wt_load_slot = {0: -1}
        for sl2, ots in loads_wt_at.items():
            for ot in ots:
                wt_load_slot[ot] = sl2
        # (ot, c) units: the 32-matmul chunk + bias add emitted together so
        # each psum tile opens and closes within one slot.
        units = [
            (
                max(
                    wt_load_slot[ot] + 1,
                    x_ready[2 * c],
                    x_ready[2 * c + 1],
                    1,
                ),
                ot,
                c,
            )
            for ot in range(OT)
            for c in range(NCH)
        ]
        units.sort()
        mms_at = {}
        taken = 0
        for slot in range(NSLOT):
            rem = len(units) - taken
            cap = rem if slot == NSLOT - 1 else -(-rem // (NSLOT - slot))
            if slot < 12:
                cap += 2
            elif slot < NSLOT - 4:
                cap += 1
            picked = []
            while len(picked) < cap and taken < len(units):
                ready, ot, c = units[taken]
                if ready > slot:
                    break
                picked.append((ot, c))
                taken += 1
            mms_at[slot] = picked

        # head: x sub-tiles 0-3 (chunks 0-1) with wt0 interleaved after
        # x0; the (now tiny) bias load rides between them
        sx_tiles = {0: load_sign_x(0)}
        wtiles = {0: load_wt(0)}
        nc.sync.dma_start(bias_sb[:], b[:, :])
        for s in (1, 2, 3):
            sx_tiles[s] = load_sign_x(s)

        xTp = {}
        osb_tiles = {}
        osb_done = {}
        store_q = {}
        for slot in range(NSLOT):
            # [SP+ACT] x sub-tile loads + eager signs (before wt so a
            # parked wt load can never block the x stream)
            for s in loads_at.get(slot, []):
                sx_tiles[s] = load_sign_x(s)
            # [SP] wt loads scheduled this slot
            for ot in loads_wt_at.get(slot, []):
                wtiles[ot] = load_wt(ot)
            # [PE] head sub-tile transposes first so chunks 0-1 are ready
            if slot == 0:
                for s in range(4):
                    c, h = divmod(s, 2)
                    if c not in xTp:
                        xTp[c] = alloc_xtp()
                    trans_x(sx_tiles.pop(s), xTp[c], h)
            # [PE] x transposes of sub-tiles loaded last slot
            for s in trs_at.get(slot, []):
                c, h = divmod(s, 2)
                if c not in xTp:
                    xTp[c] = alloc_xtp()
                trans_x(sx_tiles.pop(s), xTp[c], h)
            # [PE] matmul units ready this slot
            stores = []
            for ot, c in mms_at.get(slot, []):
                pacc = pacc_pool.tile([128, BC], F32, name="pacc", tag="pacc")
                mm_unit(wtiles[ot], xTp[c], pacc)
                cp = c // 2
                if (ot, cp) not in osb_tiles:
                    osb_tiles[(ot, cp)] = outsb.tile(
                        [128, 2 * BC], F16, tag="osb", name="osb"
                    )
                osb = osb_tiles[(ot, cp)]
                bias_add(ot, c, pacc, osb)
                n = osb_done.get((ot, cp), 0) + 1
                osb_done[(ot, cp)] = n
                if n == 2:
                    stores.append((ot, cp, osb_tiles.pop((ot, cp))))
            store_q[slot] = stores
            # [SP] deferred stores: none before STORE_START (loads own the
            # DMA engines until then), then drain the FIFO a few per slot
            if slot >= STORE_START:
                quota = 5
                for sl in sorted(store_q):
                    if sl >= slot:
                        break
                    while store_q[sl] and quota > 0:
                        ot, cp, osb = store_q[sl].pop(0)
                        store(ot, cp, osb)
                        quota -= 1
                    if quota == 0:
                        break
        for sl in sorted(store_q):
            for i, (ot, cp, osb) in enumerate(store_q[sl]):
                store(ot, cp, osb, queue=nc.scalar if i % 2 else None)

    nc.compile()
    return nc


_NC_PREP = None
_NC_CACHE = None  # main (phase B) module


def _get_modules():
    global _NC_PREP, _NC_CACHE
    if _NC_PREP is None:
        _NC_PREP = _build_prep()
    if _NC_CACHE is None:
        _NC_CACHE = _build_main()
    return _NC_PREP, _NC_CACHE


def kernel(x: np.ndarray, weight: np.ndarray, bias: np.ndarray) -> np.ndarray:
    nc_prep, nc_main = _get_modules()

    x = np.ascontiguousarray(np.asarray(x, dtype=np.float32))
    weight = np.ascontiguousarray(np.asarray(weight, dtype=np.float32))
    bias = np.ascontiguousarray(np.asarray(bias, dtype=np.float32))

    # Phase A: each core packs its 1/8 of W.
    prep_in = [
        {"w": np.ascontiguousarray(weight[i * WR:(i + 1) * WR])}
        for i in range(N_CORES)
    ]
    prep_res = run_bass_kernel_spmd(nc_prep, prep_in, list(range(N_CORES)))
    # layout-only host glue: stack the shards' out-feature blocks
    wt_global = np.ascontiguousarray(
        np.concatenate([prep_res.results[i]["swt"] for i in range(N_CORES)], axis=1)
    )

    # Phase B: batch-parallel matmul with the packed sign-W. The bias is
    # passed host-transposed [128, 32] (layout only: element (p, ot) is
    # bias[ot*128 + p]) so the device load is a single contiguous DMA.
    bias_t = np.ascontiguousarray(bias.reshape(OT, 128).T)
    main_in = [
        {
            "x": np.ascontiguousarray(x[i * Bs:(i + 1) * Bs]),
            "wt": wt_global,
            "b": bias_t,
        }
        for i in range(N_CORES)
    ]
    res = run_bass_kernel_spmd(nc_main, main_in, list(range(N_CORES)))

    out = np.empty((B, OUT), dtype=np.float32)
    for i in range(N_CORES):
        out[i * Bs:(i + 1) * Bs, :] = res.results[i]["out"].astype(np.float32).T
    return out
